# revision 7
# baseline (speedup 1.0000x reference)
"""HardClusterAssigner Trainium2 kernel.

Reference computation:
    x_emb = mean_b(einsum('bsv,hs->bvh', x, W) + b)   # [V, H]
    assignments = one_hot(argmin(-l2norm(x_emb) @ l2norm(centroids).T))

Key transformations:
  1. mean over B commutes with the linear contraction over S, the l2norm of
     the embedding is a positive per-row scale (argmin-invariant), and the
     1/B + bias fold in exactly:
         sim[v,c] = (sum_b x)[s,v] @ M[s,c] + bn[c],
         M = W.T @ cn.T,  bn = B * (b @ cn.T),  cn = l2norm(centroids)
     M/bn are x-independent and folded on the host (fp64); M ships as an
     exact fp16 hi+lo pair (~1e-7 relative), bn as an fp32 [C,1] column
     added per-partition during the slab combine.
  2. x streams as fp16 in [s, b, v] layout. Full-chunk tiles (8 KB rows)
     for t=0..5 cut the per-dma trigger count (each dma_start costs ~600ns
     serialized on the issuing HWDGE sequencer); the last two s-chunks
     stream as half tiles so the tail dependency chain is short. Consts
     ride the second HWDGE ring (Activation) so M is resident before the
     first matmul. Each tile's b-reduction runs as a 2-level halving add
     tree on the DVE (fp16 2x packed mode, contiguous slices); the PE
     contracts the remaining (s, slab) axes with fp16xfp16 products
     accumulated exactly in fp32 PSUM (M hi/lo stationary), slab segments
     overlaying the same PSUM columns.
  3. Two PSUM accumulators: psA takes t=0..6, closing early so its slab
     reduce + bias add run on the (otherwise idle) GPSIMD engine under the
     tail of the stream; psB takes t=7 only. The final combine, a GPSIMD
     partition_all_reduce(max) over the [c, v] similarity tile, and an
     is_equal one-hot produce the output directly in [C, VL] layout
     (no PE transpose); the host transposes the 16 KB result.

Sharding: V is split across the 8 cores; every stage after the split is
core-local (no collectives). Per-core time is DMA-bound: ~8.7 MB/core
streamed at the HBM roofline, with the DVE trees (~19us) and all PE work
(~15us) hidden under the stream.
"""

import sys

for _p in ("/opt/trn_rl_repo",):
    if _p not in sys.path:
        sys.path.append(_p)

from contextlib import ExitStack

import numpy as np

import concourse.bacc as bacc
import concourse.bass as bass
import concourse.bass_isa as bass_isa
import concourse.mybir as mybir
from concourse import tile
from concourse.bass_utils import run_bass_kernel_spmd

B, S, V, H, C = 64, 1024, 512, 512, 64
NCORES = 8
VL = V // NCORES  # 64 V-columns per core
P = 128
ST = S // P  # 8 s-chunks
ROW = B * VL  # 4096 fp16 elems per s-row
F16 = mybir.dt.float16
F32 = mybir.dt.float32

# x tile plan: (t, col_lo, col_hi, tree_depth, accumulator)
# t=0..5 full chunks (8KB rows, fewer triggers); t=6,7 halves (short tail).
CHUNKS = [
    (0, 0, ROW, 2, "a"),
    (1, 0, ROW, 2, "a"),
    (2, 0, ROW, 2, "a"),
    (3, 0, ROW, 2, "a"),
    (4, 0, ROW, 2, "a"),
    (5, 0, ROW, 2, "a"),
    (6, 0, ROW // 2, 2, "a"),
    (6, ROW // 2, ROW, 2, "a"),
    (7, 0, ROW // 2, 2, "b"),
    (7, ROW // 2, ROW, 2, "b"),
]
MMW = 512  # matmul segment width (psA/psB free width)

_NC_CACHE = None


def _n_matmuls(acc):
    n = 0
    for _, c0, c1, depth, a in CHUNKS:
        if a == acc:
            nb = (c1 - c0) >> depth
            n += 2 * ((nb + MMW - 1) // MMW)
    return n


def build_bass() -> bass.Bass:
    nc = bacc.Bacc("TRN2", target_bir_lowering=False)

    xs = nc.declare_dram_parameter("xs", [S, ROW], F16, isOutput=False)
    mm = nc.declare_dram_parameter("mm", [P, 2 * ST * C], F16, isOutput=False)
    bnc = nc.declare_dram_parameter("bnc", [C, 1], F32, isOutput=False)
    out = nc.declare_dram_parameter("out", [C, VL], F32, isOutput=True)

    nmm = {"a": _n_matmuls("a"), "b": _n_matmuls("b")}

    with tile.TileContext(nc) as tc, ExitStack() as ctx:
        consts = ctx.enter_context(tc.tile_pool(name="consts", bufs=1))
        xpool = ctx.enter_context(tc.tile_pool(name="xp", bufs=1))
        spool = ctx.enter_context(tc.tile_pool(name="small", bufs=1))
        psa = ctx.enter_context(tc.tile_pool(name="psa", bufs=1, space="PSUM"))
        psb = ctx.enter_context(tc.tile_pool(name="psb", bufs=1, space="PSUM"))

        msb = consts.tile([P, 2 * ST * C], F16)
        bnsb = consts.tile([C, 1], F32)
        # consts ride the Act HWDGE ring: resident well before the first
        # matmul, without serializing behind the x triggers on SP
        nc.scalar.dma_start(out=bnsb[:], in_=bnc[:])
        nc.scalar.dma_start(out=msb[:], in_=mm[:])

        psA = psa.tile([C, MMW], F32, tag="psA")
        psB = psb.tile([C, MMW], F32, tag="psB")

        # psA slab-reduce temps (Act copies PSUM->SBUF — GPSIMD can't read
        # PSUM — then GPSIMD halving-adds; all hidden under the stream tail)
        aext = spool.tile([C, MMW], F32)
        ta = spool.tile([C, MMW // 2], F32)
        sCa = spool.tile([C, VL], F32)

        xs_r = xs.rearrange("(t p) f -> t p f", p=P)
        seen = {"a": 0, "b": 0}
        for ci, (t, c0, c1, depth, acc) in enumerate(CHUNKS):
            width = c1 - c0
            xv = xpool.tile([P, width], F16, tag=f"x{ci}", name=f"xv{ci}")
            nc.sync.dma_start(out=xv[:], in_=xs_r[t][:, c0:c1])
            # halving add tree over contiguous column blocks (fp16 2x mode)
            nb = width
            for _ in range(depth):
                hb = nb // 2
                nc.vector.tensor_tensor(
                    xv[:, 0:hb], xv[:, 0:hb], xv[:, hb:nb],
                    op=mybir.AluOpType.add,
                )
                nb = hb
            # slab contraction, M_t hi/lo stationary; segments overlay the
            # same PSUM columns (sums just accumulate)
            ps = psA if acc == "a" else psB
            for li in range(2):
                for g in range(0, nb, MMW):
                    w = min(MMW, nb - g)
                    nc.tensor.matmul(
                        ps[:, 0:w],
                        msb[:, (li * ST + t) * C : (li * ST + t + 1) * C],
                        xv[:, g : g + w],
                        start=(seen[acc] == 0),
                        stop=(seen[acc] == nmm[acc] - 1),
                    )
                    seen[acc] += 1

            if acc == "a" and seen["a"] == nmm["a"]:
                # psA closed: slab-reduce + bias off the DVE, hidden under
                # the t=7 stream tail (DVE is busy with the last trees)
                nc.scalar.activation(
                    aext[:], psA[:], mybir.ActivationFunctionType.Copy
                )
                nc.gpsimd.tensor_tensor(
                    ta[:], aext[:, 0 : MMW // 2], aext[:, MMW // 2 : MMW],
                    op=mybir.AluOpType.add,
                )
                nc.gpsimd.tensor_tensor(
                    ta[:, 0 : MMW // 4], ta[:, 0 : MMW // 4],
                    ta[:, MMW // 4 : MMW // 2],
                    op=mybir.AluOpType.add,
                )
                nc.gpsimd.tensor_tensor(
                    sCa[:], ta[:, 0:VL], ta[:, VL : 2 * VL],
                    op=mybir.AluOpType.add,
                )

        # --- tail: combine psB, partition-max, one-hot, store --------------
        sCb = spool.tile([C, VL], F32)
        nc.vector.tensor_reduce(
            sCb[:],
            psB[:].rearrange("c (s v) -> c v s", s=MMW // VL),
            axis=mybir.AxisListType.X,
            op=mybir.AluOpType.add,
        )
        # sC = (sCa + bn) + sCb — bias folds into the combine for free
        sC = spool.tile([C, VL], F32)
        nc.vector.scalar_tensor_tensor(
            sC[:], sCa[:], bnsb[:], sCb[:],
            op0=mybir.AluOpType.add, op1=mybir.AluOpType.add,
        )

        mxb = spool.tile([C, VL], F32)
        nc.gpsimd.partition_all_reduce(
            mxb[:], sC[:], channels=C, reduce_op=bass_isa.ReduceOp.max
        )
        oh = spool.tile([C, VL], F32)
        nc.vector.tensor_tensor(
            oh[:], sC[:], mxb[:], op=mybir.AluOpType.is_equal
        )
        nc.sync.dma_start(out=out[:], in_=oh[:])

    nc.compile()
    return nc


def _get_nc() -> bass.Bass:
    global _NC_CACHE
    if _NC_CACHE is None:
        _NC_CACHE = build_bass()
    return _NC_CACHE


def make_in_maps(x, W, b, centroids):
    x = np.asarray(x, dtype=np.float32)
    W = np.asarray(W, dtype=np.float32)
    b = np.asarray(b, dtype=np.float32)
    centroids = np.asarray(centroids, dtype=np.float32)

    # x-independent folds, in float64, shipped as exact fp16 hi+lo pairs
    cn = centroids.astype(np.float64)
    cn /= np.linalg.norm(cn, axis=1, keepdims=True)
    M = W.astype(np.float64).T @ cn.T  # [S, C]
    bn = np.float64(B) * (b.astype(np.float64) @ cn.T)  # [C]

    Mhi = M.astype(np.float16)
    Mlo = (M - Mhi.astype(np.float64)).astype(np.float16)
    mhost = np.empty((P, 2, ST, C), np.float16)
    mhost[:, 0] = Mhi.reshape(ST, P, C).transpose(1, 0, 2)
    mhost[:, 1] = Mlo.reshape(ST, P, C).transpose(1, 0, 2)
    mhost = np.ascontiguousarray(mhost).reshape(P, 2 * ST * C)

    bnhost = np.ascontiguousarray(bn.astype(np.float32)[:, None])

    # Host layout [B,S,V] -> [S, B, VL] per core, in fp16 (cast first so the
    # transpose moves half the bytes). One pass to [S, B, V] (contiguous 1KB
    # runs), then a contiguous per-core V-slice.
    x16 = x.astype(np.float16)
    xsb = np.ascontiguousarray(x16.transpose(1, 0, 2))  # [S, B, V]
    in_maps = []
    for i in range(NCORES):
        xs_i = np.ascontiguousarray(
            xsb[:, :, i * VL : (i + 1) * VL]
        ).reshape(S, ROW)
        in_maps.append({"xs": xs_i, "mm": mhost, "bnc": bnhost})
    return in_maps


def run(inputs: dict, trace: bool = False):
    """Run on the 8 NeuronCores; returns (full_output, BassKernelResults)."""
    nc = _get_nc()
    in_maps = make_in_maps(**inputs)
    res = run_bass_kernel_spmd(nc, in_maps, list(range(NCORES)), trace=trace)
    # per-core output is [C, VL]; transpose to [VL, C] and stack over cores
    full = np.concatenate(
        [np.ascontiguousarray(r["out"].T) for r in res.results], axis=0
    )
    return full, res


def kernel(x, W, b, centroids) -> np.ndarray:
    full, _ = run({"x": x, "W": W, "b": b, "centroids": centroids})
    return full
